# revision 3
# baseline (speedup 1.0000x reference)
"""Trainium2 Bass kernel for a dense transformer block.

Reference computation (per batch item, fp32 inputs):
    h   = LN(x; ln1_g, ln1_b)
    q,k,v = per-head projections of h        (H=8 heads, D=64)
    scores = (q @ k^T) * C**-0.5, causal-masked, softmax
    o   = scores @ v, heads concatenated
    x2  = x + o @ w_proj + b_proj
    out = x2 + relu(LN(x2; ln2_g, ln2_b) @ w1 + b1) @ w2 + b2

Sharding: pure data parallel over batch. B=32 across 8 cores -> 4 batch
items per core, weights replicated, no collectives.

Key layout choices (per core):
  - LN stats computed in [t, c] layout (free-dim bn_stats), normalized
    output transposed via PE to [c, t] so the LN affine (per-c) becomes a
    per-partition tensor_scalar, and so h^T directly feeds matmuls as
    lhsT/rhs with contraction over c.
  - q^T,k^T computed head-pair-packed [128(2 heads x 64), T]; scores are
    computed transposed: scoresT[s, t] (lhsT=k^T slice, rhs=q^T), so the
    exp'd scores (bf16) are directly the lhsT of the attn@v matmul.
  - Causal mask of diagonal blocks is added inside PSUM by one extra
    accumulating matmul: strict_lower^T @ (-1e9 * I).
  - Softmax denominator l[t] comes from N=1 matmuls (expT slice @ ones),
    landing in [t-partition, head-column] layout; normalization is a
    single fused tensor_tensor multiply with a free-dim broadcast AP.
  - FFN hidden z is computed transposed [f, t] (lhsT=w1 slice, rhs=h2^T)
    so relu+b1 is one ACT op with per-partition bias and z^T directly
    feeds FFN2 as lhsT.
  - b_proj / b2 are added in PSUM via K=1 matmuls (ones_row x bias_row).

All matmuls run in bf16 (fp32 PSUM accumulation).
"""

import numpy as np
import ml_dtypes

import concourse.bass as bass
import concourse.bacc as bacc
import concourse.tile as tile
import concourse.mybir as mybir
from concourse import bass_utils

B, T, C, H, D = 32, 512, 512, 8, 64
NCORES = 8
NB = B // NCORES          # batch items per core
P = 128
NT = T // P               # 4 token tiles
NCT = C // P              # 4 channel tiles
FF = 4 * C                # 2048
NF = FF // P              # 16 hidden tiles
EPS = 1e-5
SCALE = float(C) ** -0.5
NPAIR = H // 2            # head pairs (2 heads x 64 = 128 partitions)

F32 = mybir.dt.float32
BF16 = mybir.dt.bfloat16
AF = mybir.ActivationFunctionType
OP = mybir.AluOpType
bf16 = ml_dtypes.bfloat16

_CACHE = {}


def _bcast_free(ap, reps):
    """Broadcast each free-dim element of a [P, n] AP `reps` times along a
    new innermost step-0 dim -> behaves as [P, n*reps]."""
    return bass.AP(tensor=ap.tensor, offset=ap.offset, ap=[*ap.ap, [0, reps]])


def _body(tc, io):
    nc = tc.nc
    (x_d, wq_d, wk_d, wv_d, wp_d, w1_d, w2_d, b1_d, bp_d, b2_d,
     lg1_d, lb1_d, lg2_d, lb2_d, ident_d, maskA_d, negI_d,
     ones_col_d, ones_row_d, out_d) = io

    import contextlib
    ctx = contextlib.ExitStack()
    with ctx:
        singles = ctx.enter_context(tc.tile_pool(name="singles", bufs=1))
        xp = ctx.enter_context(tc.tile_pool(name="xp", bufs=2 * NT))
        x2p = ctx.enter_context(tc.tile_pool(name="x2p", bufs=2 * NT))
        nrm = ctx.enter_context(tc.tile_pool(name="nrm", bufs=NT + 2))
        stat = ctx.enter_context(tc.tile_pool(name="stat", bufs=10))
        hTp = ctx.enter_context(tc.tile_pool(name="hTp", bufs=NCT + 2))
        qkp = ctx.enter_context(tc.tile_pool(name="qkp", bufs=2 * NPAIR + 2))
        vp = ctx.enter_context(tc.tile_pool(name="vp", bufs=NT + 2))
        expp = ctx.enter_context(tc.tile_pool(name="expp", bufs=H * NT))
        osp = ctx.enter_context(tc.tile_pool(name="osp", bufs=NT + 1))
        oTp = ctx.enter_context(tc.tile_pool(name="oTp", bufs=NCT + 1))
        h2Tp = ctx.enter_context(tc.tile_pool(name="h2Tp", bufs=NCT + 2))
        zp = ctx.enter_context(tc.tile_pool(name="zp", bufs=NF + 2))
        outp = ctx.enter_context(tc.tile_pool(name="outp", bufs=4))
        # PSUM pools: 8 banks total. mm: matmul outs + transposes (3),
        # sc: scores + l (3), op: attention-out accumulators (2).
        mmp = ctx.enter_context(tc.tile_pool(name="mmp", bufs=2, space="PSUM"))
        tpp = ctx.enter_context(tc.tile_pool(name="tpp", bufs=1, space="PSUM"))
        scp = ctx.enter_context(tc.tile_pool(name="scp", bufs=2, space="PSUM"))
        lpp = ctx.enter_context(tc.tile_pool(name="lpp", bufs=1, space="PSUM"))
        opp = ctx.enter_context(tc.tile_pool(name="opp", bufs=2, space="PSUM"))

        def load(pool, dram_ap, dtype):
            t = pool.tile(list(dram_ap.shape), dtype, tag=dram_ap.tensor.name)
            nc.sync.dma_start(out=t, in_=dram_ap)
            return t

        wq_sb = load(singles, wq_d, BF16)    # [128, NCT, 512]  (c-part, kt, h*64+d)
        wk_sb = load(singles, wk_d, BF16)
        wv_sb = load(singles, wv_d, BF16)
        wp_sb = load(singles, wp_d, BF16)    # [128, NCT, 512]
        w1_sb = load(singles, w1_d, BF16)    # [128, NCT, 2048]
        w2_sb = load(singles, w2_d, BF16)    # [128, NF, 512]
        b1_sb = load(singles, b1_d, F32)     # [128, NF]
        bp_sb = load(singles, bp_d, BF16)    # [1, 512]
        b2_sb = load(singles, b2_d, BF16)    # [1, 512]
        lg1_sb = load(singles, lg1_d, F32)   # [128, NCT]
        lb1_sb = load(singles, lb1_d, F32)
        lg2_sb = load(singles, lg2_d, F32)
        lb2_sb = load(singles, lb2_d, F32)
        ident = load(singles, ident_d, BF16)      # [128, 128]
        maskA = load(singles, maskA_d, BF16)      # [128, 128] strict lower (A[k,s]=1 if k<s)
        negI = load(singles, negI_d, BF16)        # [128, 128] -1e9 * I
        ones_col = load(singles, ones_col_d, BF16)  # [128, 1]
        ones_row = load(singles, ones_row_d, BF16)  # [1, 128]
        eps_t = singles.tile([P, 1], F32)
        nc.vector.memset(eps_t, EPS)

        def layernorm_T(x_tiles, g_sb, b_sb, hT_pool):
            """x tiles [t,c] fp32 -> per-token normalize -> PE transpose ->
            per-c affine -> returns hT tiles [c,t] bf16 (list of NCT)."""
            n_tiles = []
            for t in range(NT):
                st6 = stat.tile([P, 6], F32, tag="st6")
                nc.vector.bn_stats(out=st6, in_=x_tiles[t])
                mv = stat.tile([P, 2], F32, tag="mv")
                nc.vector.bn_aggr(out=mv, in_=st6)
                sd = stat.tile([P, 1], F32, tag="sd")
                nc.scalar.activation(out=sd, in_=mv[:, 1:2], func=AF.Sqrt,
                                     bias=eps_t)
                rstd = stat.tile([P, 1], F32, tag="rstd")
                nc.vector.reciprocal(out=rstd, in_=sd)
                n_t = nrm.tile([P, T], BF16, tag="n")
                nc.vector.tensor_scalar(out=n_t, in0=x_tiles[t],
                                        scalar1=mv[:, 0:1], scalar2=rstd,
                                        op0=OP.subtract, op1=OP.mult)
                n_tiles.append(n_t)
            hT = []
            for i in range(NCT):
                tp = tpp.tile([P, T], BF16, tag="tps")
                for t in range(NT):
                    nc.tensor.transpose(tp[:, P * t:P * (t + 1)],
                                        n_tiles[t][:, P * i:P * (i + 1)], ident)
                h_i = hT_pool.tile([P, T], BF16)
                nc.vector.tensor_scalar(out=h_i, in0=tp,
                                        scalar1=g_sb[:, i:i + 1],
                                        scalar2=b_sb[:, i:i + 1],
                                        op0=OP.mult, op1=OP.add)
                hT.append(h_i)
            return hT

        for b in range(NB):
            # ---- LN1 ----
            x_tiles = []
            for t in range(NT):
                x_t = xp.tile([P, C], F32)
                nc.sync.dma_start(out=x_t, in_=x_d[b, P * t:P * (t + 1), :])
                x_tiles.append(x_t)
            hT = layernorm_T(x_tiles, lg1_sb, lb1_sb, hTp)

            # ---- QKV ----
            qT, kT = [], []
            for pr in range(NPAIR):
                qps = mmp.tile([P, T], F32, tag="mm")
                for kt in range(NCT):
                    nc.tensor.matmul(qps, wq_sb[:, kt, P * pr:P * (pr + 1)],
                                     hT[kt], start=(kt == 0), stop=(kt == NCT - 1))
                q_sb = qkp.tile([P, T], BF16, tag="qk")
                # fold the C**-0.5 score scale into q
                nc.scalar.activation(out=q_sb, in_=qps, func=AF.Copy, scale=SCALE)
                qT.append(q_sb)
                kps = mmp.tile([P, T], F32, tag="mm")
                for kt in range(NCT):
                    nc.tensor.matmul(kps, wk_sb[:, kt, P * pr:P * (pr + 1)],
                                     hT[kt], start=(kt == 0), stop=(kt == NCT - 1))
                k_sb = qkp.tile([P, T], BF16, tag="qk")
                nc.vector.tensor_copy(out=k_sb, in_=kps)
                kT.append(k_sb)
            v_sb = []
            for st in range(NT):
                vps = mmp.tile([P, C], F32, tag="mm")
                for kt in range(NCT):
                    nc.tensor.matmul(vps, hT[kt][:, P * st:P * (st + 1)],
                                     wv_sb[:, kt, :], start=(kt == 0),
                                     stop=(kt == NCT - 1))
                v_t = vp.tile([P, C], BF16)
                nc.vector.tensor_copy(out=v_t, in_=vps)
                v_sb.append(v_t)

            # ---- scores^T + exp (per head, per s-tile) ----
            expT = [[None] * NT for _ in range(H)]
            for h in range(H):
                pr, off = divmod(h, 2)
                off *= D
                for i in range(NT):
                    w = T - P * i  # valid t range: [P*i, T)
                    sc = scp.tile([P, T], F32, tag="sc")
                    nc.tensor.matmul(sc[:, P * i:],
                                     kT[pr][off:off + D, P * i:P * (i + 1)],
                                     qT[pr][off:off + D, P * i:],
                                     start=True, stop=False,
                                     skip_group_check=True)
                    # causal mask on the diagonal block: += A^T @ (-1e9 I)
                    nc.tensor.matmul(sc[:, P * i:P * (i + 1)], maskA, negI,
                                     start=False, stop=True,
                                     skip_group_check=True)
                    e_t = expp.tile([P, T], BF16)
                    nc.scalar.activation(out=e_t[:, P * i:], in_=sc[:, P * i:],
                                         func=AF.Exp)
                    expT[h][i] = e_t

            # ---- attention out (o = softmax @ v), t-tile major ----
            o_sb = []
            for m in range(NT):
                ops_ = opp.tile([P, C], F32, tag="op")
                lps = lpp.tile([P, H], F32, tag="l")
                for h in range(H):
                    for i in range(m + 1):
                        lhs = expT[h][i][:, P * m:P * (m + 1)]
                        nc.tensor.matmul(ops_[:, D * h:D * (h + 1)], lhs,
                                         v_sb[i][:, D * h:D * (h + 1)],
                                         start=(i == 0), stop=(i == m),
                                         skip_group_check=True)
                        nc.tensor.matmul(lps[:, h:h + 1], lhs, ones_col,
                                         start=(i == 0), stop=(i == m),
                                         skip_group_check=True)
                linv = stat.tile([P, H], F32, tag="linv")
                nc.vector.reciprocal(out=linv, in_=lps)
                o_t = osp.tile([P, C], BF16)
                nc.vector.tensor_tensor(out=o_t, in0=ops_,
                                        in1=_bcast_free(linv[:], D), op=OP.mult)
                o_sb.append(o_t)

            # ---- transpose o ----
            oT = []
            for i in range(NCT):
                tp = tpp.tile([P, T], BF16, tag="tps")
                for m in range(NT):
                    nc.tensor.transpose(tp[:, P * m:P * (m + 1)],
                                        o_sb[m][:, P * i:P * (i + 1)], ident)
                oT_i = oTp.tile([P, T], BF16)
                nc.vector.tensor_copy(out=oT_i, in_=tp)
                oT.append(oT_i)

            # ---- proj + residual ----
            x2_tiles = []
            for m in range(NT):
                yps = mmp.tile([P, C], F32, tag="mm")
                for kt in range(NCT):
                    nc.tensor.matmul(yps, oT[kt][:, P * m:P * (m + 1)],
                                     wp_sb[:, kt, :], start=(kt == 0), stop=False,
                                     skip_group_check=True)
                nc.tensor.matmul(yps, ones_row, bp_sb, start=False, stop=True,
                                 skip_group_check=True)
                x2_t = x2p.tile([P, C], F32)
                nc.vector.tensor_tensor(out=x2_t, in0=yps, in1=x_tiles[m],
                                        op=OP.add)
                x2_tiles.append(x2_t)

            # ---- LN2 ----
            h2T = layernorm_T(x2_tiles, lg2_sb, lb2_sb, h2Tp)

            # ---- FFN1 (z^T = relu(w1^T @ h2 + b1)) ----
            zT = []
            for j in range(NF):
                zps = mmp.tile([P, T], F32, tag="mm")
                for kt in range(NCT):
                    nc.tensor.matmul(zps, w1_sb[:, kt, P * j:P * (j + 1)],
                                     h2T[kt], start=(kt == 0), stop=(kt == NCT - 1))
                z_j = zp.tile([P, T], BF16)
                nc.scalar.activation(out=z_j, in_=zps, func=AF.Relu,
                                     bias=b1_sb[:, j:j + 1])
                zT.append(z_j)

            # ---- FFN2 + residual ----
            for m in range(NT):
                fps = mmp.tile([P, C], F32, tag="mm")
                for kt in range(NF):
                    nc.tensor.matmul(fps, zT[kt][:, P * m:P * (m + 1)],
                                     w2_sb[:, kt, :], start=(kt == 0), stop=False,
                                     skip_group_check=True)
                nc.tensor.matmul(fps, ones_row, b2_sb, start=False, stop=True,
                                 skip_group_check=True)
                o_t = outp.tile([P, C], F32)
                nc.vector.tensor_tensor(out=o_t, in0=fps, in1=x2_tiles[m],
                                        op=OP.add)
                nc.sync.dma_start(out=out_d[b, P * m:P * (m + 1), :], in_=o_t)


def _build():
    nc = bacc.Bacc("TRN2", target_bir_lowering=False, debug=False,
                   num_devices=NCORES)
    d = nc.dram_tensor
    io = (
        d("x", [NB, T, C], F32, kind="ExternalInput").ap(),
        d("wq", [P, NCT, C], BF16, kind="ExternalInput").ap(),
        d("wk", [P, NCT, C], BF16, kind="ExternalInput").ap(),
        d("wv", [P, NCT, C], BF16, kind="ExternalInput").ap(),
        d("wp", [P, NCT, C], BF16, kind="ExternalInput").ap(),
        d("w1", [P, NCT, FF], BF16, kind="ExternalInput").ap(),
        d("w2", [P, NF, C], BF16, kind="ExternalInput").ap(),
        d("b1", [P, NF], F32, kind="ExternalInput").ap(),
        d("bp", [1, C], BF16, kind="ExternalInput").ap(),
        d("b2", [1, C], BF16, kind="ExternalInput").ap(),
        d("lg1", [P, NCT], F32, kind="ExternalInput").ap(),
        d("lb1", [P, NCT], F32, kind="ExternalInput").ap(),
        d("lg2", [P, NCT], F32, kind="ExternalInput").ap(),
        d("lb2", [P, NCT], F32, kind="ExternalInput").ap(),
        d("ident", [P, P], BF16, kind="ExternalInput").ap(),
        d("maskA", [P, P], BF16, kind="ExternalInput").ap(),
        d("negI", [P, P], BF16, kind="ExternalInput").ap(),
        d("ones_col", [P, 1], BF16, kind="ExternalInput").ap(),
        d("ones_row", [1, P], BF16, kind="ExternalInput").ap(),
        d("out", [NB, T, C], F32, kind="ExternalOutput").ap(),
    )
    with tile.TileContext(nc) as tc:
        _body(tc, io)
    nc.compile()
    return nc


def _ktile(w, part):
    """[K, M] -> [128, K//128, M] with K = 128*kt + p."""
    k, m = w.shape
    return np.ascontiguousarray(
        w.reshape(k // part, part, m).transpose(1, 0, 2))


def _col(v, part):
    """[N] -> [128, N//128] with n = 128*j + p."""
    return np.ascontiguousarray(v.reshape(-1, part).T)


def kernel(**inputs):
    if "nc" not in _CACHE:
        _CACHE["nc"] = _build()
    nc = _CACHE["nc"]

    f32 = lambda a: np.asarray(a, np.float32)
    x = f32(inputs["x"])
    wq = f32(inputs["wq"]).transpose(1, 0, 2).reshape(C, C)   # [c, h*D+d]
    wk = f32(inputs["wk"]).transpose(1, 0, 2).reshape(C, C)
    wv = f32(inputs["wv"]).transpose(1, 0, 2).reshape(C, C)

    common = {
        "wq": _ktile(wq, P).astype(bf16),
        "wk": _ktile(wk, P).astype(bf16),
        "wv": _ktile(wv, P).astype(bf16),
        "wp": _ktile(f32(inputs["w_proj"]), P).astype(bf16),
        "w1": _ktile(f32(inputs["w1"]), P).astype(bf16),
        "w2": _ktile(f32(inputs["w2"]), P).astype(bf16),
        "b1": _col(f32(inputs["b1"]), P),
        "bp": f32(inputs["b_proj"]).reshape(1, C).astype(bf16),
        "b2": f32(inputs["b2"]).reshape(1, C).astype(bf16),
        "lg1": _col(f32(inputs["ln1_g"]), P),
        "lb1": _col(f32(inputs["ln1_b"]), P),
        "lg2": _col(f32(inputs["ln2_g"]), P),
        "lb2": _col(f32(inputs["ln2_b"]), P),
        "ident": np.eye(P, dtype=bf16),
        "maskA": np.triu(np.ones((P, P), np.float32), 1).astype(bf16),
        "negI": (-1e9 * np.eye(P, dtype=np.float32)).astype(bf16),
        "ones_col": np.ones((P, 1), bf16),
        "ones_row": np.ones((1, P), bf16),
    }
    in_maps = [dict(common, x=np.ascontiguousarray(x[c * NB:(c + 1) * NB]))
               for c in range(NCORES)]

    res = bass_utils.run_bass_kernel_spmd(nc, in_maps,
                                          core_ids=list(range(NCORES)))
    _CACHE["last_result"] = res
    return np.concatenate([r["out"] for r in res.results], axis=0)


# revision 4
# speedup vs baseline: 8233.2834x; 8233.2834x over previous
"""Trainium2 Bass kernel for a dense transformer block.

Reference computation (per batch item, fp32 inputs):
    h   = LN(x; ln1_g, ln1_b)
    q,k,v = per-head projections of h        (H=8 heads, D=64)
    scores = (q @ k^T) * C**-0.5, causal-masked, softmax
    o   = scores @ v, heads concatenated
    x2  = x + o @ w_proj + b_proj
    out = x2 + relu(LN(x2; ln2_g, ln2_b) @ w1 + b1) @ w2 + b2

Sharding: pure data parallel over batch. B=32 across 8 cores -> 4 batch
items per core, weights replicated, no collectives.

Key layout choices (per core):
  - LN stats computed in [t, c] layout (free-dim bn_stats), normalized
    output transposed via PE to [c, t] so the LN affine (per-c) becomes a
    per-partition tensor_scalar, and so h^T directly feeds matmuls as
    lhsT/rhs with contraction over c.
  - q^T,k^T computed head-pair-packed [128(2 heads x 64), T]; scores are
    computed transposed: scoresT[s, t] (lhsT=k^T slice, rhs=q^T), so the
    exp'd scores (bf16) are directly the lhsT of the attn@v matmul.
  - Causal mask of diagonal blocks is added inside PSUM by one extra
    accumulating matmul: strict_lower^T @ (-1e9 * I).
  - Softmax denominator l[t] comes from N=1 matmuls (expT slice @ ones),
    landing in [t-partition, head-column] layout; normalization is a
    single fused tensor_tensor multiply with a free-dim broadcast AP.
  - FFN hidden z is computed transposed [f, t] (lhsT=w1 slice, rhs=h2^T)
    so relu+b1 is one ACT op with per-partition bias and z^T directly
    feeds FFN2 as lhsT.
  - b_proj / b2 are added in PSUM via K=1 matmuls (ones_row x bias_row).

All matmuls run in bf16 (fp32 PSUM accumulation).
"""

import numpy as np
import ml_dtypes

import concourse.bass as bass
import concourse.bacc as bacc
import concourse.tile as tile
import concourse.mybir as mybir
from concourse import bass_utils

B, T, C, H, D = 32, 512, 512, 8, 64
NCORES = 8
NB = B // NCORES          # batch items per core
P = 128
NT = T // P               # 4 token tiles
NCT = C // P              # 4 channel tiles
FF = 4 * C                # 2048
NF = FF // P              # 16 hidden tiles
EPS = 1e-5
SCALE = float(C) ** -0.5
NPAIR = H // 2            # head pairs (2 heads x 64 = 128 partitions)

F32 = mybir.dt.float32
BF16 = mybir.dt.bfloat16
AF = mybir.ActivationFunctionType
OP = mybir.AluOpType
bf16 = ml_dtypes.bfloat16

_CACHE = {}


def _bcast_free(ap, reps):
    """Broadcast each free-dim element of a [P, n] AP `reps` times along a
    new innermost step-0 dim -> behaves as [P, n*reps]."""
    return bass.AP(tensor=ap.tensor, offset=ap.offset, ap=[*ap.ap, [0, reps]])


def _body(tc, io):
    nc = tc.nc
    (x_d, wq_d, wk_d, wv_d, wp_d, w1_d, w2_d, b1_d, bp_d, b2_d,
     lg1_d, lb1_d, lg2_d, lb2_d, ident_d, maskA_d, negI_d,
     ones_col_d, ones_row_d, out_d) = io

    import contextlib
    ctx = contextlib.ExitStack()
    with ctx:
        singles = ctx.enter_context(tc.tile_pool(name="singles", bufs=1))
        xp = ctx.enter_context(tc.tile_pool(name="xp", bufs=2 * NT))
        x2p = ctx.enter_context(tc.tile_pool(name="x2p", bufs=2 * NT))
        nrm = ctx.enter_context(tc.tile_pool(name="nrm", bufs=NT + 2))
        stat = ctx.enter_context(tc.tile_pool(name="stat", bufs=10))
        hTp = ctx.enter_context(tc.tile_pool(name="hTp", bufs=NCT + 2))
        qkp = ctx.enter_context(tc.tile_pool(name="qkp", bufs=2 * NPAIR + 2))
        vp = ctx.enter_context(tc.tile_pool(name="vp", bufs=NT + 2))
        expp = ctx.enter_context(tc.tile_pool(name="expp", bufs=H * NT))
        osp = ctx.enter_context(tc.tile_pool(name="osp", bufs=NT + 1))
        oTp = ctx.enter_context(tc.tile_pool(name="oTp", bufs=NCT + 1))
        h2Tp = ctx.enter_context(tc.tile_pool(name="h2Tp", bufs=NCT + 2))
        zp = ctx.enter_context(tc.tile_pool(name="zp", bufs=NF + 2))
        outp = ctx.enter_context(tc.tile_pool(name="outp", bufs=4))
        # PSUM pools: 8 banks total. mm: matmul outs + transposes (3),
        # sc: scores + l (3), op: attention-out accumulators (2).
        mmp = ctx.enter_context(tc.tile_pool(name="mmp", bufs=2, space="PSUM"))
        tpp = ctx.enter_context(tc.tile_pool(name="tpp", bufs=1, space="PSUM"))
        scp = ctx.enter_context(tc.tile_pool(name="scp", bufs=2, space="PSUM"))
        lpp = ctx.enter_context(tc.tile_pool(name="lpp", bufs=1, space="PSUM"))
        opp = ctx.enter_context(tc.tile_pool(name="opp", bufs=2, space="PSUM"))

        def load(pool, dram_ap, dtype):
            t = pool.tile(list(dram_ap.shape), dtype, tag=dram_ap.tensor.name)
            nc.sync.dma_start(out=t, in_=dram_ap)
            return t

        wq_sb = load(singles, wq_d, BF16)    # [128, NCT, 512]  (c-part, kt, h*64+d)
        wk_sb = load(singles, wk_d, BF16)
        wv_sb = load(singles, wv_d, BF16)
        wp_sb = load(singles, wp_d, BF16)    # [128, NCT, 512]
        w1_sb = load(singles, w1_d, BF16)    # [128, NCT, 2048]
        w2_sb = load(singles, w2_d, BF16)    # [128, NF, 512]
        b1_sb = load(singles, b1_d, F32)     # [128, NF]
        bp_sb = load(singles, bp_d, BF16)    # [1, 512]
        b2_sb = load(singles, b2_d, BF16)    # [1, 512]
        lg1_sb = load(singles, lg1_d, F32)   # [128, NCT]
        lb1_sb = load(singles, lb1_d, F32)
        lg2_sb = load(singles, lg2_d, F32)
        lb2_sb = load(singles, lb2_d, F32)
        ident = load(singles, ident_d, BF16)      # [128, 128]
        maskA = load(singles, maskA_d, BF16)      # [128, 128] strict lower (A[k,s]=1 if k<s)
        negI = load(singles, negI_d, BF16)        # [128, 128] -1e9 * I
        ones_col = load(singles, ones_col_d, BF16)  # [128, 1]
        ones_row = load(singles, ones_row_d, BF16)  # [1, 128]
        eps_t = singles.tile([P, 1], F32)
        nc.vector.memset(eps_t, EPS)

        def layernorm_T(x_tiles, g_sb, b_sb, hT_pool):
            """x tiles [t,c] fp32 -> per-token normalize -> PE transpose ->
            per-c affine -> returns hT tiles [c,t] bf16 (list of NCT)."""
            n_tiles = []
            for t in range(NT):
                st6 = stat.tile([P, 6], F32, tag="st6")
                nc.vector.bn_stats(out=st6, in_=x_tiles[t])
                mv = stat.tile([P, 2], F32, tag="mv")
                nc.vector.bn_aggr(out=mv, in_=st6)
                sd = stat.tile([P, 1], F32, tag="sd")
                nc.scalar.activation(out=sd, in_=mv[:, 1:2], func=AF.Sqrt,
                                     bias=eps_t)
                rstd = stat.tile([P, 1], F32, tag="rstd")
                nc.vector.reciprocal(out=rstd, in_=sd)
                n_t = nrm.tile([P, T], BF16, tag="n")
                nc.vector.tensor_scalar(out=n_t, in0=x_tiles[t],
                                        scalar1=mv[:, 0:1], scalar2=rstd,
                                        op0=OP.subtract, op1=OP.mult)
                n_tiles.append(n_t)
            hT = []
            for i in range(NCT):
                tp = tpp.tile([P, T], BF16, tag="tps")
                for t in range(NT):
                    nc.tensor.transpose(tp[:, P * t:P * (t + 1)],
                                        n_tiles[t][:, P * i:P * (i + 1)], ident)
                h_i = hT_pool.tile([P, T], BF16)
                nc.vector.tensor_scalar(out=h_i, in0=tp,
                                        scalar1=g_sb[:, i:i + 1],
                                        scalar2=b_sb[:, i:i + 1],
                                        op0=OP.mult, op1=OP.add)
                hT.append(h_i)
            return hT

        for b in range(NB):
            # ---- LN1 ----
            x_tiles = []
            for t in range(NT):
                x_t = xp.tile([P, C], F32)
                nc.sync.dma_start(out=x_t, in_=x_d[b, P * t:P * (t + 1), :])
                x_tiles.append(x_t)
            hT = layernorm_T(x_tiles, lg1_sb, lb1_sb, hTp)

            # ---- QKV ----
            qT, kT = [], []
            for pr in range(NPAIR):
                qps = mmp.tile([P, T], F32, tag="mm")
                for kt in range(NCT):
                    nc.tensor.matmul(qps, wq_sb[:, kt, P * pr:P * (pr + 1)],
                                     hT[kt], start=(kt == 0), stop=(kt == NCT - 1))
                q_sb = qkp.tile([P, T], BF16, tag="qk")
                # fold the C**-0.5 score scale into q
                nc.scalar.activation(out=q_sb, in_=qps, func=AF.Copy, scale=SCALE)
                qT.append(q_sb)
                kps = mmp.tile([P, T], F32, tag="mm")
                for kt in range(NCT):
                    nc.tensor.matmul(kps, wk_sb[:, kt, P * pr:P * (pr + 1)],
                                     hT[kt], start=(kt == 0), stop=(kt == NCT - 1))
                k_sb = qkp.tile([P, T], BF16, tag="qk")
                nc.vector.tensor_copy(out=k_sb, in_=kps)
                kT.append(k_sb)
            v_sb = []
            for st in range(NT):
                vps = mmp.tile([P, C], F32, tag="mm")
                for kt in range(NCT):
                    nc.tensor.matmul(vps, hT[kt][:, P * st:P * (st + 1)],
                                     wv_sb[:, kt, :], start=(kt == 0),
                                     stop=(kt == NCT - 1))
                v_t = vp.tile([P, C], BF16)
                nc.vector.tensor_copy(out=v_t, in_=vps)
                v_sb.append(v_t)

            # ---- scores^T + exp (per head, per s-tile) ----
            expT = [[None] * NT for _ in range(H)]
            for h in range(H):
                pr, off = divmod(h, 2)
                off *= D
                for i in range(NT):
                    w = T - P * i  # valid t range: [P*i, T)
                    sc = scp.tile([P, T], F32, tag="sc")
                    nc.tensor.matmul(sc[:, P * i:],
                                     kT[pr][off:off + D, P * i:P * (i + 1)],
                                     qT[pr][off:off + D, P * i:],
                                     start=True, stop=False,
                                     skip_group_check=True)
                    # causal mask on the diagonal block: += A^T @ (-1e9 I)
                    nc.tensor.matmul(sc[:, P * i:P * (i + 1)], maskA, negI,
                                     start=False, stop=True,
                                     skip_group_check=True)
                    e_t = expp.tile([P, T], BF16)
                    nc.scalar.activation(out=e_t[:, P * i:], in_=sc[:, P * i:],
                                         func=AF.Exp)
                    expT[h][i] = e_t

            # ---- attention out (o = softmax @ v), t-tile major ----
            o_sb = []
            for m in range(NT):
                ops_ = opp.tile([P, C], F32, tag="op")
                lps = lpp.tile([P, H], F32, tag="l")
                for h in range(H):
                    for i in range(m + 1):
                        lhs = expT[h][i][:, P * m:P * (m + 1)]
                        nc.tensor.matmul(ops_[:, D * h:D * (h + 1)], lhs,
                                         v_sb[i][:, D * h:D * (h + 1)],
                                         start=(i == 0), stop=(i == m),
                                         skip_group_check=True)
                        nc.tensor.matmul(lps[:, h:h + 1], lhs, ones_col,
                                         start=(i == 0), stop=(i == m),
                                         skip_group_check=True)
                linv = stat.tile([P, H], F32, tag="linv")
                nc.vector.reciprocal(out=linv, in_=lps)
                o_t = osp.tile([P, C], BF16)
                nc.vector.tensor_tensor(out=o_t, in0=ops_,
                                        in1=_bcast_free(linv[:], D), op=OP.mult)
                o_sb.append(o_t)

            # ---- transpose o ----
            oT = []
            for i in range(NCT):
                tp = tpp.tile([P, T], BF16, tag="tps")
                for m in range(NT):
                    nc.tensor.transpose(tp[:, P * m:P * (m + 1)],
                                        o_sb[m][:, P * i:P * (i + 1)], ident)
                oT_i = oTp.tile([P, T], BF16)
                nc.vector.tensor_copy(out=oT_i, in_=tp)
                oT.append(oT_i)

            # ---- proj + residual ----
            x2_tiles = []
            for m in range(NT):
                yps = mmp.tile([P, C], F32, tag="mm")
                for kt in range(NCT):
                    nc.tensor.matmul(yps, oT[kt][:, P * m:P * (m + 1)],
                                     wp_sb[:, kt, :], start=(kt == 0), stop=False,
                                     skip_group_check=True)
                nc.tensor.matmul(yps, ones_row, bp_sb, start=False, stop=True,
                                 skip_group_check=True)
                x2_t = x2p.tile([P, C], F32)
                nc.vector.tensor_tensor(out=x2_t, in0=yps, in1=x_tiles[m],
                                        op=OP.add)
                x2_tiles.append(x2_t)

            # ---- LN2 ----
            h2T = layernorm_T(x2_tiles, lg2_sb, lb2_sb, h2Tp)

            # ---- FFN1 (z^T = relu(w1^T @ h2 + b1)) ----
            zT = []
            for j in range(NF):
                zps = mmp.tile([P, T], F32, tag="mm")
                for kt in range(NCT):
                    nc.tensor.matmul(zps, w1_sb[:, kt, P * j:P * (j + 1)],
                                     h2T[kt], start=(kt == 0), stop=(kt == NCT - 1))
                z_j = zp.tile([P, T], BF16)
                nc.scalar.activation(out=z_j, in_=zps, func=AF.Relu,
                                     bias=b1_sb[:, j:j + 1])
                zT.append(z_j)

            # ---- FFN2 + residual ----
            for m in range(NT):
                fps = mmp.tile([P, C], F32, tag="mm")
                for kt in range(NF):
                    nc.tensor.matmul(fps, zT[kt][:, P * m:P * (m + 1)],
                                     w2_sb[:, kt, :], start=(kt == 0), stop=False,
                                     skip_group_check=True)
                nc.tensor.matmul(fps, ones_row, b2_sb, start=False, stop=True,
                                 skip_group_check=True)
                o_t = outp.tile([P, C], F32)
                nc.vector.tensor_tensor(out=o_t, in0=fps, in1=x2_tiles[m],
                                        op=OP.add)
                nc.sync.dma_start(out=out_d[b, P * m:P * (m + 1), :], in_=o_t)


def _build():
    nc = bacc.Bacc("TRN2", target_bir_lowering=False, debug=False,
                   num_devices=NCORES)
    d = nc.dram_tensor
    io = (
        d("x", [NB, T, C], F32, kind="ExternalInput").ap(),
        d("wq", [P, NCT, C], BF16, kind="ExternalInput").ap(),
        d("wk", [P, NCT, C], BF16, kind="ExternalInput").ap(),
        d("wv", [P, NCT, C], BF16, kind="ExternalInput").ap(),
        d("wp", [P, NCT, C], BF16, kind="ExternalInput").ap(),
        d("w1", [P, NCT, FF], BF16, kind="ExternalInput").ap(),
        d("w2", [P, NF, C], BF16, kind="ExternalInput").ap(),
        d("b1", [P, NF], F32, kind="ExternalInput").ap(),
        d("bp", [1, C], BF16, kind="ExternalInput").ap(),
        d("b2", [1, C], BF16, kind="ExternalInput").ap(),
        d("lg1", [P, NCT], F32, kind="ExternalInput").ap(),
        d("lb1", [P, NCT], F32, kind="ExternalInput").ap(),
        d("lg2", [P, NCT], F32, kind="ExternalInput").ap(),
        d("lb2", [P, NCT], F32, kind="ExternalInput").ap(),
        d("ident", [P, P], BF16, kind="ExternalInput").ap(),
        d("maskA", [P, P], BF16, kind="ExternalInput").ap(),
        d("negI", [P, P], BF16, kind="ExternalInput").ap(),
        d("ones_col", [P, 1], BF16, kind="ExternalInput").ap(),
        d("ones_row", [1, P], BF16, kind="ExternalInput").ap(),
        d("out", [NB, T, C], F32, kind="ExternalOutput").ap(),
    )
    with tile.TileContext(nc) as tc:
        _body(tc, io)
    nc.compile()
    return nc


def _ktile(w, part):
    """[K, M] -> [128, K//128, M] with K = 128*kt + p."""
    k, m = w.shape
    return np.ascontiguousarray(
        w.reshape(k // part, part, m).transpose(1, 0, 2))


def _col(v, part):
    """[N] -> [128, N//128] with n = 128*j + p."""
    return np.ascontiguousarray(v.reshape(-1, part).T)


def kernel(**inputs):
    if "nc" not in _CACHE:
        _CACHE["nc"] = _build()
    nc = _CACHE["nc"]

    f32 = lambda a: np.asarray(a, np.float32)
    x = f32(inputs["x"])
    wq = f32(inputs["wq"]).transpose(1, 0, 2).reshape(C, C)   # [c, h*D+d]
    wk = f32(inputs["wk"]).transpose(1, 0, 2).reshape(C, C)
    wv = f32(inputs["wv"]).transpose(1, 0, 2).reshape(C, C)

    common = {
        "wq": _ktile(wq, P).astype(bf16),
        "wk": _ktile(wk, P).astype(bf16),
        "wv": _ktile(wv, P).astype(bf16),
        "wp": _ktile(f32(inputs["w_proj"]), P).astype(bf16),
        "w1": _ktile(f32(inputs["w1"]), P).astype(bf16),
        "w2": _ktile(f32(inputs["w2"]), P).astype(bf16),
        "b1": _col(f32(inputs["b1"]), P),
        "bp": f32(inputs["b_proj"]).reshape(1, C).astype(bf16),
        "b2": f32(inputs["b2"]).reshape(1, C).astype(bf16),
        "lg1": _col(f32(inputs["ln1_g"]), P),
        "lb1": _col(f32(inputs["ln1_b"]), P),
        "lg2": _col(f32(inputs["ln2_g"]), P),
        "lb2": _col(f32(inputs["ln2_b"]), P),
        "ident": np.eye(P, dtype=bf16),
        "maskA": np.triu(np.ones((P, P), np.float32), 1).astype(bf16),
        "negI": (-1e9 * np.eye(P, dtype=np.float32)).astype(bf16),
        "ones_col": np.ones((P, 1), bf16),
        "ones_row": np.ones((1, P), bf16),
    }
    in_maps = [dict(common, x=np.ascontiguousarray(x[c * NB:(c + 1) * NB]))
               for c in range(NCORES)]

    res = bass_utils.run_bass_kernel_spmd(nc, in_maps,
                                          core_ids=list(range(NCORES)),
                                          trace=_CACHE.get("trace", False))
    _CACHE["last_result"] = res
    return np.concatenate([r["out"] for r in res.results], axis=0)
